# revision 47
# baseline (speedup 1.0000x reference)
"""GQA attention kernel for Trainium2, 8-core SPMD.

Sharding: core c = 2*b + g handles batch b (of 4) and head-group g (of 2):
8 of 16 q-heads, 2 of 4 kv-heads.  Each core computes its partial
out^T = (attn_out @ wo_g^T)^T in transposed space (no on-chip transposes);
the host adds the two group partials per batch and transposes back.

The kernel is Tensor-exec bound (~N/2.4 ns per matmul of moving size N,
any dtype), so everything that is not a real GEMM is kept OFF the PE:
  - causal mask: gpsimd.affine_select on the exp tiles (off the PE).
  - softmax denominator: DVE adds of exp tiles + ONE all-ones [128,128]
    matmul per q-tile that partition-reduces AND broadcasts in one shot,
    then reciprocal_approx_fast (DVE) and one DVE multiply to normalize.
  - RoPE pair-swap: SBUF->SBUF DMA partition swap (no PE perm-matmul).
  - fp16 storage everywhere; PSUM accumulation stays fp32.
  - x resident in SBUF (loaded once); Q and O never leave SBUF.
  - software pipeline: score matmuls run LAG pair-units ahead of the
    p-consuming matmuls in the in-order Tensor queue; Q-projection of
    head h+1 fills remaining PE gaps during attention of head h.

Everything on-chip is computed in transposed orientation:
  Q^T/K^T: [head_dim(part), T]   scores^T: [kt(part), qt]   O^T: [d(part), qt]
RoPE is handled by permuting wq/wk rows on the host to an
[evens | odds] layout (scores are invariant to a shared d-permutation).
"""

import math
import numpy as np

B, T, C = 4, 2048, 2048
N_HEAD, N_KV_HEAD, HD = 16, 4, 128
N_CORES = 8
SCALE = 1.0 / math.sqrt(HD)

_PROG = {}
_LAST_IN_MAPS = None


def _build_program():
    from contextlib import ExitStack
    import concourse.bacc as bacc
    import concourse.mybir as mybir
    import concourse.tile as tile

    f16 = mybir.dt.float16
    f32 = mybir.dt.float32
    Exp = mybir.ActivationFunctionType.Exp

    nc = bacc.Bacc(None, target_bir_lowering=False)
    xH = nc.declare_dram_parameter("xH", [128, 16, T], f16, isOutput=False)
    wqH = nc.declare_dram_parameter("wqH", [8, 128, 16, 128], f16, isOutput=False)
    wkH = nc.declare_dram_parameter("wkH", [128, 16, 256], f16, isOutput=False)
    wvH = nc.declare_dram_parameter("wvH", [128, 16, 256], f16, isOutput=False)
    woH = nc.declare_dram_parameter("woH", [128, 8, T], f16, isOutput=False)
    cos2H = nc.declare_dram_parameter("cos2", [128, T], f16, isOutput=False)
    sin2H = nc.declare_dram_parameter("sin2", [128, T], f16, isOutput=False)
    out = nc.declare_dram_parameter("out", [C, T], f16, isOutput=True)

    with tile.TileContext(nc) as tc, nc.allow_low_precision(
        reason="fp16 storage with fp32 PSUM accumulation; tolerance is 2e-2"
    ), ExitStack() as top:
        consts = top.enter_context(tc.tile_pool(name="consts", bufs=1))
        cs2 = consts.tile([128, T], f16)
        sn2 = consts.tile([128, T], f16)
        ones_sq = consts.tile([128, 128], f16)
        nc.vector.memset(ones_sq, 1.0)

        data = top.enter_context(tc.tile_pool(name="data", bufs=1))
        x_sb = data.tile([128, 16, T], f16)
        K_sb = data.tile([128, 2, T], f16)
        V_sb = data.tile([128, 16, 256], f16)
        O_sb = data.tile([128, 8, T], f16)
        wo_sb = data.tile([128, 8, T], f16)

        # pools shared by K-rope (KV pass) and Q-proj/rope (attention era)
        qraws = top.enter_context(tc.tile_pool(name="qraws", bufs=4))
        ropes = top.enter_context(tc.tile_pool(name="ropes", bufs=2))
        ps_aux = top.enter_context(tc.tile_pool(name="ps_aux", bufs=1, space="PSUM"))
        wqp = top.enter_context(tc.tile_pool(name="wqp", bufs=2))
        qsbp = top.enter_context(tc.tile_pool(name="qsbp", bufs=2))
        pwork = top.enter_context(tc.tile_pool(name="pwork", bufs=3))
        dwork = top.enter_context(tc.tile_pool(name="dwork", bufs=1))
        dfold = top.enter_context(tc.tile_pool(name="dfold", bufs=1))
        rbcp = top.enter_context(tc.tile_pool(name="rbcp", bufs=1))

        def emit_rope(raw_f16, dst, tsl):
            # dst = raw*cs2 + swap_halves(raw)*sn2; the half-swap is a
            # partition-swapped SBUF->SBUF DMA copy (DMA engines are idle)
            sw = qraws.tile([128, 512], f16, tag="sw", name="sw")
            nc.sync.dma_start(out=sw[0:64, :], in_=raw_f16[64:128, :])
            nc.sync.dma_start(out=sw[64:128, :], in_=raw_f16[0:64, :])
            ta = ropes.tile([128, 512], f16, tag="ta", name="ta")
            tb = ropes.tile([128, 512], f16, tag="tb", name="tb")
            nc.vector.tensor_mul(ta, raw_f16, cs2[:, tsl])
            nc.vector.tensor_mul(tb, sw, sn2[:, tsl])
            nc.vector.tensor_add(dst, ta, tb)

        q_tiles = {}
        wq_tiles = {}

        def issue_wq(h):
            # issued ~a full head ahead of use: the [128,16,128] transfer is
            # ~128 descriptors on one queue and must not race its consumers
            wq_sb = wqp.tile([128, 16, 128], f16, tag="wq", name=f"wq{h}")
            wq_tiles[h] = wq_sb
            nc.sync.dma_start(out=wq_sb, in_=wqH[h])

        def qproj_ops(h):
            """Closure list computing Q_sb for head h (proj + rope)."""
            ops = []
            wq_sb = wq_tiles.pop(h)
            q_sb = qsbp.tile([128, T], f16, tag="q", name=f"q{h}")
            q_tiles[h] = q_sb
            for t4 in range(4):
                tsl = slice(t4 * 512, (t4 + 1) * 512)
                q_ps = ps_aux.tile([128, 512], f32, tag="aux", name="qp")
                for ci in range(16):
                    ops.append(lambda q_ps=q_ps, ci=ci, tsl=tsl: nc.tensor.matmul(
                        q_ps, wq_sb[:, ci, :], x_sb[:, ci, tsl],
                        start=(ci == 0), stop=(ci == 15),
                    ))
                def rope_q(q_ps=q_ps, tsl=tsl):
                    raw = qraws.tile([128, 512], f16, tag="raw", name="raw")
                    nc.scalar.copy(raw, q_ps)
                    emit_rope(raw, q_sb[:, tsl], tsl)
                ops.append(rope_q)
            return ops

        # head 0's projection is interleaved into the KV pass (late blocks,
        # after x has mostly arrived, so it never head-of-line blocks)
        issue_wq(0)
        issue_wq(1)
        ops0 = qproj_ops(0)
        kv_filler = [[], [], ops0[0:34], ops0[34:]]

        # ---- phase 1: K/V projections + K RoPE (x resident in SBUF) ----
        with ExitStack() as kv_stack:
            wkv = kv_stack.enter_context(tc.tile_pool(name="wkv", bufs=1))
            wk_sb = wkv.tile([128, 16, 256], f16)
            wv_sb = wkv.tile([128, 16, 256], f16)
            # DMA order matters (per-queue FIFO): wk/wv gate the first
            # matmuls, x gates the whole pass; consts are needed only at
            # the first rope; wo not until phase 3 (emitted much later).
            nc.sync.dma_start(out=wk_sb, in_=wkH[:])
            nc.sync.dma_start(out=wv_sb, in_=wvH[:])
            for ci in range(16):
                nc.sync.dma_start(out=x_sb[:, ci, :], in_=xH[:, ci, :])
            nc.sync.dma_start(out=cs2, in_=cos2H[:])
            nc.sync.dma_start(out=sn2, in_=sin2H[:])
            ps_k = kv_stack.enter_context(tc.tile_pool(name="ps_k", bufs=2, space="PSUM"))
            ps_v = kv_stack.enter_context(tc.tile_pool(name="ps_v", bufs=4, space="PSUM"))
            for t4 in range(4):
                tsl = slice(t4 * 512, (t4 + 1) * 512)
                k_ps = [ps_k.tile([128, 512], f32, tag="kps", name=f"kps{i}")
                        for i in range(2)]
                v_ps = [ps_v.tile([128, 256], f32, tag="vps", name=f"vps{i}")
                        for i in range(4)]
                for ci in range(16):
                    for kv in range(2):
                        nc.tensor.matmul(
                            k_ps[kv], wk_sb[:, ci, kv * 128:(kv + 1) * 128],
                            x_sb[:, ci, tsl], start=(ci == 0), stop=(ci == 15),
                        )
                    for sub in range(4):
                        nc.tensor.matmul(
                            v_ps[sub],
                            x_sb[:, ci, t4 * 512 + sub * 128:t4 * 512 + (sub + 1) * 128],
                            wv_sb[:, ci, :], start=(ci == 0), stop=(ci == 15),
                        )
                for sub in range(4):
                    nc.scalar.copy(V_sb[:, t4 * 4 + sub, :], v_ps[sub])
                for kv in range(2):
                    raw = qraws.tile([128, 512], f16, tag="raw", name="raw")
                    nc.scalar.copy(raw, k_ps[kv])
                    emit_rope(raw, K_sb[:, kv, tsl], tsl)
                for op in kv_filler[t4]:
                    op()

        # ---- attention era: software-pipelined per head ----
        # Score matmuls run LAG pair-units ahead of the p-consuming matmuls
        # in the (in-order) Tensor queue, so the scalar engine's exp stream
        # runs back-to-back.  Scores for two adjacent 128-wide k-blocks
        # share one [128,1024] PSUM tile -> one wide exp.  Causal masking
        # happens on the exp tiles via gpsimd affine_select (off the PE).
        with ExitStack() as at_stack:
            ps_s = at_stack.enter_context(tc.tile_pool(name="ps_s", bufs=2, space="PSUM"))
            ps_o = at_stack.enter_context(tc.tile_pool(name="ps_o", bufs=2, space="PSUM"))
            ps_den = at_stack.enter_context(tc.tile_pool(name="ps_den", bufs=1, space="PSUM"))
            LAG = 2

            def emit_attn(h, filler):
                kv = h // 4
                q_sb = q_tiles.pop(h)
                units = []
                for qj in range(4):
                    nk = 4 * (qj + 1)
                    for kp in range(nk // 2):
                        units.append((qj, 2 * kp, 2 * kp + 1))
                n = len(units)
                p_tiles = [None] * n
                qj_state = {}

                def emit_score(i):
                    qj, k0, k1 = units[i]
                    s2 = ps_s.tile([128, 1024], f32, tag="s", name="s2")
                    for j, ki in enumerate((k0, k1)):
                        # diagonal blocks (r>0): fully-masked columns qt <
                        # r*128 are never computed; exp of the stale PSUM
                        # there is finite garbage that affine_select zeroes.
                        r = max(ki - 4 * qj, 0)
                        nc.tensor.matmul(
                            s2[:, j * 512 + r * 128:(j + 1) * 512],
                            K_sb[:, kv, ki * 128:(ki + 1) * 128],
                            q_sb[:, qj * 512 + r * 128:(qj + 1) * 512],
                            start=True, stop=True,
                        )
                    p2 = pwork.tile([128, 1024], f16, tag="p", name="p2")
                    nc.scalar.activation(p2, s2, Exp, scale=SCALE)
                    for j, ki in enumerate((k0, k1)):
                        if ki >= 4 * qj:  # diagonal block: zero masked region
                            nc.gpsimd.affine_select(
                                out=p2[:, j * 512:(j + 1) * 512],
                                in_=p2[:, j * 512:(j + 1) * 512],
                                pattern=[[1, 512]],
                                compare_op=mybir.AluOpType.is_ge, fill=0.0,
                                base=qj * 512 - ki * 128, channel_multiplier=-1,
                            )
                    p_tiles[i] = p2

                pending = []

                def emit_normalize():
                    # deferred by one unit so the den-reduce matmul never
                    # head-of-line blocks the Tensor queue on the DVE fold
                    o_ps, den_f, qsl = pending.pop(0)
                    den_ps = ps_den.tile([128, 512], f32, tag="dn", name="den_ps")
                    nc.tensor.matmul(den_ps, ones_sq, den_f)
                    rbc = rbcp.tile([128, 512], f32, tag="rbc", name="rbc")
                    nc.vector.reciprocal_approx_fast(out=rbc, in_=den_ps)
                    nc.vector.tensor_mul(O_sb[:, h, qsl], o_ps, rbc)

                def emit_consume(i):
                    qj, k0, k1 = units[i]
                    qsl = slice(qj * 512, (qj + 1) * 512)
                    nk = 4 * (qj + 1)
                    if k0 == 0:
                        qj_state[qj] = (
                            ps_o.tile([128, 512], f32, tag="o", name="o_ps"),
                            dwork.tile([128, 1024], f16, tag="dw", name="den_w"),
                        )
                    o_ps, den_w = qj_state[qj]
                    p2 = p_tiles[i]
                    p_tiles[i] = None
                    if k0 == 0:
                        nc.vector.tensor_copy(den_w, p2)
                    else:
                        nc.vector.tensor_add(den_w, den_w, p2)
                    for j, ki in enumerate((k0, k1)):
                        r = max(ki - 4 * qj, 0)
                        nc.tensor.matmul(
                            o_ps[:, r * 128:512],
                            V_sb[:, ki, kv * 128:(kv + 1) * 128],
                            p2[:, j * 512 + r * 128:(j + 1) * 512],
                            start=(ki == 0), stop=(ki == nk - 1),
                        )
                    if pending:
                        emit_normalize()
                    if k1 == nk - 1:
                        den_f = dfold.tile([128, 512], f16, tag="df", name="den_f")
                        nc.vector.tensor_add(den_f, den_w[:, 0:512], den_w[:, 512:1024])
                        pending.append((o_ps, den_f, qsl))

                for i in range(n + LAG):
                    if i < n:
                        emit_score(i)
                        for _ in range(2):
                            if filler:
                                filler.pop(0)()
                    if i >= LAG:
                        emit_consume(i - LAG)
                        for _ in range(2):
                            if filler:
                                filler.pop(0)()
                while pending:
                    emit_normalize()

            nc.sync.dma_start(out=wo_sb, in_=woH[:])
            for h in range(8):
                if h + 2 <= 7:
                    issue_wq(h + 2)
                filler = qproj_ops(h + 1) if h < 7 else []
                emit_attn(h, filler)
                for op in filler:
                    op()

        # ---- phase 3: output projection (transposed partials) ----
        with ExitStack() as ph3:
            outsb = ph3.enter_context(tc.tile_pool(name="outsb", bufs=4))
            ps_out = ph3.enter_context(tc.tile_pool(name="ps_out", bufs=6, space="PSUM"))
            for tj in range(4):
                tsl = slice(tj * 512, (tj + 1) * 512)
                for e in range(16):
                    op_ = ps_out.tile([128, 512], f32, tag="op", name="op")
                    for hh in range(8):
                        nc.tensor.matmul(
                            op_, wo_sb[:, hh, e * 128:(e + 1) * 128],
                            O_sb[:, hh, tsl], start=(hh == 0), stop=(hh == 7),
                        )
                    ob = outsb.tile([128, 512], f16, tag="ob", name="ob")
                    nc.vector.tensor_copy(ob, op_)
                    if tj == 3 and e >= 14:
                        # last tiles: 4-way split so the final transfer does
                        # not serialize 128 descriptors on one queue
                        for sl in range(4):
                            nc.sync.dma_start(
                                out=out[e * 128 + 32 * sl:e * 128 + 32 * (sl + 1), tsl],
                                in_=ob[32 * sl:32 * (sl + 1)],
                            )
                    else:
                        nc.sync.dma_start(out=out[e * 128:(e + 1) * 128, tsl], in_=ob)

    nc.compile()
    return nc


def _get_program():
    if "nc" not in _PROG:
        _PROG["nc"] = _build_program()
    return _PROG["nc"]


def kernel(x, wq, wk, wv, wo, rope_cos, rope_sin):
    from concourse.bass_utils import run_bass_kernel_spmd

    nc = _get_program()
    x = np.asarray(x, dtype=np.float32)
    wq = np.asarray(wq, dtype=np.float32)
    wk = np.asarray(wk, dtype=np.float32)
    wv = np.asarray(wv, dtype=np.float32)
    wo = np.asarray(wo, dtype=np.float32)
    cosT = np.asarray(rope_cos, dtype=np.float32).T  # [64, T]
    sinT = np.asarray(rope_sin, dtype=np.float32).T

    # even/odd -> [evens | odds] permutation of each head's rows of wq/wk
    perm = np.concatenate([np.arange(0, HD, 2), np.arange(1, HD, 2)])
    wq_p = wq.reshape(N_HEAD, HD, C)[:, perm, :]          # [16, 128, C]
    wk_p = wk.reshape(N_KV_HEAD, HD, C)[:, perm, :]       # [4, 128, C]
    wv_r = wv.reshape(N_KV_HEAD, HD, C)                   # [4, 128, C]

    cos2 = np.concatenate([cosT, cosT], axis=0).astype(np.float16)
    sin2 = np.concatenate([-sinT, sinT], axis=0).astype(np.float16)

    def part_major(a):  # [rows(c=n*128+p), m] -> [128(p), n, m]
        rows, m = a.shape
        return np.ascontiguousarray(
            a.reshape(rows // 128, 128, m).transpose(1, 0, 2))

    in_maps = []
    for core in range(N_CORES):
        b, g = core // 2, core % 2
        xT = x[b].T.astype(np.float16)                     # [C, T]
        wq_g = wq_p[8 * g:8 * g + 8]                       # [8, 128, C]
        wqHa = np.stack([part_major(wq_g[hl].T.astype(np.float16))
                         for hl in range(8)])              # [8, 128, 16, 128]
        wkHa = np.concatenate(
            [part_major(wk_p[2 * g + kv].T.astype(np.float16))
             for kv in range(2)], axis=2)                  # [128, 16, 256]
        wvHa = np.concatenate(
            [part_major(wv_r[2 * g + kv].T.astype(np.float16))
             for kv in range(2)], axis=2)
        wo_g = wo[:, 1024 * g:1024 * (g + 1)]              # [C(e), 1024(hd)]
        woHa = part_major(wo_g.T.astype(np.float16))
        # woHa: rows = hd = hl*128 + p -> [128(p), 8(hl), C(e)]
        in_maps.append({
            "xH": part_major(xT),
            "wqH": wqHa,
            "wkH": wkHa,
            "wvH": wvHa,
            "woH": woHa,
            "cos2": cos2,
            "sin2": sin2,
        })

    global _LAST_IN_MAPS
    _LAST_IN_MAPS = in_maps
    res = run_bass_kernel_spmd(nc, in_maps, list(range(N_CORES))).results
    outp = np.empty((B, T, C), dtype=np.float32)
    for b in range(B):
        outp[b] = (res[2 * b]["out"].astype(np.float32)
                   + res[2 * b + 1]["out"].astype(np.float32)).T
    return outp
